# revision 4
# baseline (speedup 1.0000x reference)
"""PolyGNN Trainium2 kernel.

Strategy (8 NeuronCores, B=4):
  - core c works on batch c // 2 (pair-replicated in v1).
  - GCN runs in transposed layout x^T [C, N] per batch, N=1024 columns with a
    4-column cyclic halo so the ring-adjacency matvec adj@x becomes 8
    accumulating PE matmuls with free-dim-shifted rhs windows (adjacent is a
    banded circulant: vertex i <-> i+-1..4 mod N, weights 1/8).
  - Bilinear CNN-feature interpolation: index/weight math on DVE in a
    [128, 8]-chunked vertex layout, rows gathered from a host-pretransposed
    DRAM table [HW, 256] bf16 via dma_gather, weighted sum on DVE, then PE
    transposes into the [C, N] GCN input.
"""

import sys

sys.path.insert(0, "/opt/trn_rl_repo")
sys.path.insert(0, "/root/.axon_site/_ro/trn_rl_repo")

import numpy as np
import ml_dtypes

B, N, FH, FW, C = 4, 1024, 128, 128, 130
SDIM, STEPS = 128, 3
HAL = 4           # cyclic halo columns on each side
XW = N + 2 * HAL  # 1032
ELEM = 256        # padded gather row length (bf16 -> 512B, %256B ok)
NCORES = 8
P = 128

_cache = {}


def _build_program(n_diag_blocks_uniform: bool):
    import concourse.bass as bass
    import concourse.bacc as bacc
    import concourse.tile as tile
    from concourse import mybir

    f32 = mybir.dt.float32
    bf16 = mybir.dt.bfloat16
    i16 = mybir.dt.int16
    i32 = mybir.dt.int32
    ALU = mybir.AluOpType
    ACTF = mybir.ActivationFunctionType

    DS = [-4, -3, -2, -1, 1, 2, 3, 4]

    # weight blob layout (must mirror host packing)
    # per step: gcn0 [W1(128), W2(128 or 8x128)], 12x res [W1, W2...], gcn7 [W1(32), W2(32...)]
    nwb = 1 if n_diag_blocks_uniform else 8

    def gconv_meta():
        # (dout, has_rest, relu, residual)
        metas = [(SDIM, True, False, False)]
        for _ in range(6):
            metas.append((SDIM, False, True, False))
            metas.append((SDIM, False, True, True))
        metas.append((32, False, False, False))
        return metas

    metas = gconv_meta()
    # compute main-blob column offsets
    offs = []
    cur = 0
    for s in range(STEPS):
        step_offs = []
        for dout, has_rest, relu, resid in metas:
            w1o = cur
            cur += dout
            w2o = [cur + (i if not n_diag_blocks_uniform else 0) * dout for i in range(8)]
            cur += dout * nwb
            step_offs.append((w1o, w2o))
        offs.append(step_offs)
    NW = cur
    # rest blob: per step W1rest(128) + W2rest blocks
    roffs = []
    cur = 0
    for s in range(STEPS):
        r1 = cur
        cur += SDIM
        r2 = [cur + (i if not n_diag_blocks_uniform else 0) * SDIM for i in range(8)]
        cur += SDIM * nwb
        roffs.append((r1, r2))
    NR = cur

    nc = bacc.Bacc(None, target_bir_lowering=False, debug=False)

    conv_d = nc.dram_tensor("conv", [FH * FW, ELEM], bf16, kind="ExternalInput")
    xy0_d = nc.dram_tensor("xy0", [P, 16], f32, kind="ExternalInput")
    hull0_d = nc.dram_tensor("hull0", [2, N], f32, kind="ExternalInput")
    wmain_d = nc.dram_tensor("wmain", [P, NW], bf16, kind="ExternalInput")
    wrest_d = nc.dram_tensor("wrest", [64, NR], bf16, kind="ExternalInput")
    wfc_d = nc.dram_tensor("wfc", [32, 2 * STEPS], bf16, kind="ExternalInput")
    bias_d = nc.dram_tensor("bias", [P, 14 * STEPS], f32, kind="ExternalInput")
    bfc_d = nc.dram_tensor("bfc", [2, STEPS], f32, kind="ExternalInput")
    ident_d = nc.dram_tensor("ident", [P, P], bf16, kind="ExternalInput")
    identf_d = nc.dram_tensor("identf", [P, P], f32, kind="ExternalInput")
    r16_d = nc.dram_tensor("r16", [16, P], f32, kind="ExternalInput")
    preds_d = nc.dram_tensor("preds", [STEPS, 2, N], f32, kind="ExternalOutput")

    with tile.TileContext(nc) as tc:
        with (
            tc.tile_pool(name="persist", bufs=1) as pp,
            tc.tile_pool(name="interp", bufs=2) as ip,
            tc.tile_pool(name="gpool", bufs=1) as gp,
            tc.tile_pool(name="xpool", bufs=4) as xp,
            tc.tile_pool(name="psmm", bufs=4, space="PSUM") as psmm,
            tc.tile_pool(name="pstr", bufs=2, space="PSUM") as pstr,
        ):
            # ---- load persistent data
            wmain_t = pp.tile([P, NW], bf16)
            nc.sync.dma_start(wmain_t[:], wmain_d[:])
            wrest_t = pp.tile([64, NR], bf16)
            nc.sync.dma_start(wrest_t[:], wrest_d[:])
            wfc_t = pp.tile([32, 2 * STEPS], bf16)
            nc.sync.dma_start(wfc_t[:], wfc_d[:])
            bias_t = pp.tile([P, 14 * STEPS], f32)
            nc.sync.dma_start(bias_t[:], bias_d[:])
            bfc_t = pp.tile([2, STEPS], f32)
            nc.sync.dma_start(bfc_t[:], bfc_d[:])
            ident_t = pp.tile([P, P], bf16)
            nc.sync.dma_start(ident_t[:], ident_d[:])
            identf_t = pp.tile([P, P], f32)
            nc.sync.dma_start(identf_t[:], identf_d[:])
            r16_t = pp.tile([16, P], f32)
            nc.sync.dma_start(r16_t[:], r16_d[:])
            xy0_t = pp.tile([P, 16], f32)
            nc.sync.dma_start(xy0_t[:], xy0_d[:])
            hull_t = pp.tile([2, N], f32)
            nc.sync.dma_start(hull_t[:], hull0_d[:])

            for s in range(STEPS):
                # ================= interp: S = Xs/Ys in [128, (j,c)] chunk layout
                if s == 0:
                    s_t = xy0_t
                else:
                    hc_t = ip.tile([P, 16], f32, tag="hc")
                    for j in range(8):
                        trp = pstr.tile([P, 2], f32, space="PSUM", tag="pstr")
                        nc.tensor.transpose(
                            trp[:], hull_t[:, 128 * j : 128 * j + 128],
                            identf_t[0:2, 0:2],
                        )
                        nc.vector.tensor_copy(hc_t[:, 2 * j : 2 * j + 2], trp[:])
                    s_t = ip.tile([P, 16], f32, tag="s")
                    nc.vector.tensor_scalar(
                        out=s_t[:], in0=hc_t[:], scalar1=128.0, scalar2=None,
                        op0=ALU.mult,
                    )

                # floor & frac (rounding-mode agnostic)
                ri_t = ip.tile([P, 16], i32, tag="ri")
                nc.vector.tensor_copy(ri_t[:], s_t[:])
                rf_t = ip.tile([P, 16], f32, tag="rf")
                nc.vector.tensor_copy(rf_t[:], ri_t[:])
                cm_t = ip.tile([P, 16], f32, tag="cm")
                nc.vector.tensor_tensor(out=cm_t[:], in0=s_t[:], in1=rf_t[:], op=ALU.is_lt)
                f0_t = ip.tile([P, 16], f32, tag="f0")
                nc.vector.tensor_tensor(out=f0_t[:], in0=rf_t[:], in1=cm_t[:], op=ALU.subtract)
                fr_t = ip.tile([P, 16], f32, tag="fr")
                nc.vector.tensor_tensor(out=fr_t[:], in0=s_t[:], in1=f0_t[:], op=ALU.subtract)
                f0c_t = ip.tile([P, 16], f32, tag="f0c")
                nc.vector.tensor_scalar(
                    out=f0c_t[:], in0=f0_t[:], scalar1=0.0, scalar2=127.0,
                    op0=ALU.max, op1=ALU.min,
                )
                t1_t = ip.tile([P, 16], f32, tag="t1")
                nc.vector.tensor_scalar(
                    out=t1_t[:], in0=f0_t[:], scalar1=1.0, scalar2=0.0,
                    op0=ALU.add, op1=ALU.max,
                )
                f1c_t = ip.tile([P, 16], f32, tag="f1c")
                nc.vector.tensor_scalar(
                    out=f1c_t[:], in0=t1_t[:], scalar1=127.0, scalar2=None, op0=ALU.min,
                )
                om_t = ip.tile([P, 16], f32, tag="om")
                nc.vector.tensor_scalar(
                    out=om_t[:], in0=fr_t[:], scalar1=1.0, scalar2=-1.0,
                    op0=ALU.subtract, op1=ALU.mult,
                )

                def xy(t):
                    v = t[:].rearrange("p (j c) -> p j c", c=2)
                    return v[:, :, 0], v[:, :, 1]

                frx, fry = xy(fr_t)
                omx, omy = xy(om_t)
                f0cx, f0cy = xy(f0c_t)
                f1cx, f1cy = xy(f1c_t)

                # bilinear weights, bf16 [128, 8] each
                w_ts = []
                for wi, (ax, ay) in enumerate(
                    [(omx, omy), (omx, fry), (frx, omy), (frx, fry)]
                ):
                    w_t = ip.tile([P, 8], bf16, tag=f"w{wi}")
                    nc.vector.tensor_tensor(out=w_t[:], in0=ax, in1=ay, op=ALU.mult)
                    w_ts.append(w_t)

                # gather row indices (fp32), nbr-major [128, (i, j)]
                a0_t = ip.tile([P, 8], f32, tag="a0")
                nc.vector.tensor_scalar(
                    out=a0_t[:], in0=f0cx, scalar1=float(FW), scalar2=None, op0=ALU.mult
                )
                a1_t = ip.tile([P, 8], f32, tag="a1")
                nc.vector.tensor_scalar(
                    out=a1_t[:], in0=f1cx, scalar1=float(FW), scalar2=None, op0=ALU.mult
                )
                idxall_t = ip.tile([P, 32], f32, tag="idxall")
                for i, (ax, ay) in enumerate(
                    [(a0_t, f0cy), (a0_t, f1cy), (a1_t, f0cy), (a1_t, f1cy)]
                ):
                    axap = ax[:] if isinstance(ax, type(a0_t)) else ax
                    nc.vector.tensor_tensor(
                        out=idxall_t[:, 8 * i : 8 * i + 8], in0=axap, in1=ay, op=ALU.add
                    )

                # ---- convert to dma_gather wrapped-idx layout [16, 64] x4, replicated
                tps = pstr.tile([32, P], f32, space="PSUM", tag="pstr")
                nc.tensor.transpose(tps[:], idxall_t[:], identf_t[:])
                u_t = ip.tile([32, P], f32, tag="u")
                nc.vector.tensor_copy(u_t[:], tps[:])
                idx16f_t = ip.tile([16, 256], f32, tag="idx16f")
                for a in range(8):
                    uap = pstr.tile([16, 32], f32, space="PSUM", tag="pstr")
                    nc.tensor.transpose(
                        uap[:], u_t[:, 16 * a : 16 * a + 16], identf_t[0:32, 0:32]
                    )
                    dst = bass.AP(
                        idx16f_t.tensor,
                        idx16f_t[:].offset + a,
                        [[idx16f_t[:].ap[0][0], 16], [64, 4], [8, 8]],
                    )
                    nc.vector.tensor_copy(dst, uap[:])
                repp = pstr.tile([P, 256], f32, space="PSUM", tag="pstr")
                nc.tensor.matmul(
                    repp[:], lhsT=r16_t[:], rhs=idx16f_t[:], start=True, stop=True
                )
                idx16_t = ip.tile([P, 256], i16, tag="idx16")
                nc.vector.tensor_copy(idx16_t[:], repp[:])

                # ---- gathers
                g_ts = []
                for i in range(4):
                    g_t = gp.tile([P, 8, ELEM], bf16, tag=f"g{i}")
                    nc.gpsimd.dma_gather(
                        g_t[:],
                        conv_d[:],
                        idx16_t[:, 64 * i : 64 * i + 64],
                        N,
                        N,
                        ELEM,
                    )
                    g_ts.append(g_t)

                # ---- weighted sum -> cnn [128, 8, 130] bf16
                def wbc(w_t):
                    ap = w_t[:]
                    return bass.AP(ap.tensor, ap.offset, [[ap.ap[0][0], P], [1, 8], [0, C]])

                m_ts = []
                for i in range(4):
                    m_t = ip.tile([P, 8, C], bf16, tag=f"m{i % 2}_{i // 2}")
                    nc.vector.tensor_tensor(
                        out=m_t[:], in0=g_ts[i][:, :, 0:C], in1=wbc(w_ts[i]), op=ALU.mult
                    )
                    m_ts.append(m_t)
                s01_t = ip.tile([P, 8, C], bf16, tag="s01")
                nc.vector.tensor_tensor(out=s01_t[:], in0=m_ts[0][:], in1=m_ts[1][:], op=ALU.add)
                s23_t = ip.tile([P, 8, C], bf16, tag="s23")
                nc.vector.tensor_tensor(out=s23_t[:], in0=m_ts[2][:], in1=m_ts[3][:], op=ALU.add)
                cnn_t = ip.tile([P, 8, C], bf16, tag="cnn")
                nc.vector.tensor_tensor(out=cnn_t[:], in0=s01_t[:], in1=s23_t[:], op=ALU.add)

                # ---- transpose into GCN input x^T
                x_m = xp.tile([P, XW], bf16, tag="xm")
                x_r = xp.tile([64, XW], bf16, tag="xr")
                nc.vector.memset(x_r[:], 0.0)
                for j in range(8):
                    tmp = pstr.tile([P, P], bf16, space="PSUM", tag="pstr")
                    nc.tensor.transpose(tmp[:], cnn_t[:, j, 0:128], ident_t[:])
                    nc.scalar.copy(x_m[:, HAL + 128 * j : HAL + 128 * j + 128], tmp[:])
                    tmp2 = pstr.tile([2, P], bf16, space="PSUM", tag="pstr")
                    nc.tensor.transpose(tmp2[:], cnn_t[:, j, 128:130], ident_t[:])
                    nc.vector.tensor_copy(
                        x_r[0:2, HAL + 128 * j : HAL + 128 * j + 128], tmp2[:]
                    )
                nc.vector.tensor_copy(x_r[32:34, HAL : HAL + N], hull_t[:])
                # halos
                nc.vector.tensor_copy(x_m[:, 0:HAL], x_m[:, N : N + HAL])
                nc.vector.tensor_copy(x_m[:, N + HAL : N + 2 * HAL], x_m[:, HAL : 2 * HAL])
                nc.vector.tensor_copy(x_r[:, 0:HAL], x_r[:, N : N + HAL])
                nc.vector.tensor_copy(x_r[:, N + HAL : N + 2 * HAL], x_r[:, HAL : 2 * HAL])

                # ---- GCN
                xres_m = None
                for g, (dout, has_rest, relu, resid) in enumerate(metas):
                    w1o, w2o = offs[s][g]
                    y_m = xp.tile([dout, XW] if dout != SDIM else [P, XW], bf16, tag="xm")
                    for h in range(2):
                        base = HAL + 512 * h
                        pshalf = psmm.tile([dout, 512], f32, space="PSUM", tag="psum")
                        # accumulation group: W1, [W1rest], 8x shifted W2 [+rest], [residual]
                        mms = [(wmain_t[:, w1o : w1o + dout], x_m[:, base : base + 512])]
                        if has_rest:
                            r1, r2 = roffs[s]
                            mms.append(
                                (wrest_t[:, r1 : r1 + SDIM], x_r[:, base : base + 512])
                            )
                        for di, d in enumerate(DS):
                            mms.append(
                                (
                                    wmain_t[:, w2o[di] : w2o[di] + dout],
                                    x_m[:, base + d : base + d + 512],
                                )
                            )
                            if has_rest:
                                r1, r2 = roffs[s]
                                mms.append(
                                    (
                                        wrest_t[:, r2[di] : r2[di] + SDIM],
                                        x_r[:, base + d : base + d + 512],
                                    )
                                )
                        if resid:
                            mms.append((ident_t[:], xres_m[:, base : base + 512]))
                        for mi, (lhsT_ap, rhs_ap) in enumerate(mms):
                            nc.tensor.matmul(
                                pshalf[:],
                                lhsT=lhsT_ap,
                                rhs=rhs_ap,
                                start=(mi == 0),
                                stop=(mi == len(mms) - 1),
                            )
                        # evacuate with bias (+relu)
                        nc.scalar.activation(
                            y_m[0:dout, base : base + 512],
                            pshalf[:],
                            ACTF.Relu if relu else ACTF.Identity,
                            bias=bias_t[0:dout, 14 * s + g : 14 * s + g + 1],
                            scale=1.0,
                        )
                    # halos
                    nc.vector.tensor_copy(y_m[0:dout, 0:HAL], y_m[0:dout, N : N + HAL])
                    nc.vector.tensor_copy(
                        y_m[0:dout, N + HAL : N + 2 * HAL], y_m[0:dout, HAL : 2 * HAL]
                    )
                    if g + 1 < len(metas) and metas[g + 1][3]:
                        xres_m = x_m  # input of gconv_a feeds residual of gconv_b
                    x_m = y_m

                # ---- fc + hull update + snapshot
                for h in range(2):
                    base = HAL + 512 * h
                    pf = psmm.tile([2, 512], f32, space="PSUM", tag="psum")
                    nc.tensor.matmul(
                        pf[:],
                        lhsT=wfc_t[:, 2 * s : 2 * s + 2],
                        rhs=x_m[0:32, base : base + 512],
                        start=True,
                        stop=True,
                    )
                    sh_t = ip.tile([2, 512], f32, tag="sh")
                    nc.scalar.activation(
                        sh_t[:], pf[:], ACTF.Identity,
                        bias=bfc_t[:, s : s + 1], scale=1.0,
                    )
                    nc.vector.tensor_tensor(
                        out=hull_t[:, 512 * h : 512 * h + 512],
                        in0=hull_t[:, 512 * h : 512 * h + 512],
                        in1=sh_t[:],
                        op=ALU.add,
                    )
                nc.sync.dma_start(preds_d[s], hull_t[:])

    nc.compile()
    return nc


_RESID_FIX_NOTE = """residual handled via identity matmul"""


def _host_prep(tg2, original_hull, binary_hull, bbox, params):
    """Build per-core input maps."""
    tg2 = np.asarray(tg2, np.float32)
    original_hull = np.asarray(original_hull, np.float32)
    binary_hull = np.asarray(binary_hull, np.float32)
    bbox = np.asarray(bbox, np.float32)

    ident = np.eye(P, dtype=ml_dtypes.bfloat16)
    identf = np.eye(P, dtype=np.float32)
    r16 = np.zeros((16, P), np.float32)
    for q in range(P):
        r16[q % 16, q] = 1.0

    in_maps = []
    for core in range(NCORES):
        b = core // 2
        conv = np.zeros((FH * FW, ELEM), ml_dtypes.bfloat16)
        conv[:, 0:C] = (
            tg2[b].transpose(1, 2, 0).reshape(FH * FW, C).astype(ml_dtypes.bfloat16)
        )
        h, w = bbox[b, 3], bbox[b, 2]
        s0 = np.empty((N, 2), np.float32)
        s0[:, 0] = original_hull[b, :, 0] / h * FH
        s0[:, 1] = original_hull[b, :, 1] / w * FW
        # chunk layout [p, (j, c)]
        xy0 = s0.reshape(8, 128, 2).transpose(1, 0, 2).reshape(P, 16).copy()
        hull0 = np.ascontiguousarray(binary_hull[b].T)
        in_maps.append(
            {
                "conv": conv,
                "xy0": xy0,
                "hull0": hull0,
                "ident": ident,
                "identf": identf,
                "r16": r16,
            }
        )
    return in_maps


def _pack_weights(params, cds_uniform, cval, cds):
    """Pack per-step GCN weights into blobs. Returns dict of arrays."""
    nwb = 1 if cds_uniform else 8
    wmain_blocks = []
    wrest_blocks = []
    wfc_cols = []
    bias_cols = []
    bfc_cols = []
    for s in range(STEPS):
        p = params[s]
        gconvs = [p["gcn0"]]
        for pa, pb in p["res"]:
            gconvs.append(pa)
            gconvs.append(pb)
        gconvs.append(p["gcn7"])
        for gi, (W1, b1, W2, b2) in enumerate(gconvs):
            W1 = np.asarray(W1, np.float32)
            W2 = np.asarray(W2, np.float32)
            b = np.asarray(b1, np.float32) + np.asarray(b2, np.float32)
            dout = W1.shape[1]
            if gi == 0:
                def _rest(Wsub):
                    r = np.zeros((64, Wsub.shape[1]), np.float32)
                    r[0:2] = Wsub[0:2]    # cnn channels 128, 129
                    r[32:34] = Wsub[2:4]  # hull x, y rows
                    return r
                if cds_uniform:
                    # spread-3: adj@x = c*(box3(s) - x), s = x<<-3 + x + x<<3
                    wmain_blocks.append(W1[0:128] - cval * W2[0:128])
                    wrest_blocks.append(_rest(W1[128:132] - cval * W2[128:132]))
                    wmain_blocks.append(W2[0:128] * cval)
                    wrest_blocks.append(_rest(W2[128:132] * cval))
                else:
                    wmain_blocks.append(W1[0:128])
                    wrest_blocks.append(_rest(W1[128:132]))
                    for cd in cds:
                        wmain_blocks.append(W2[0:128] * cd)
                        wrest_blocks.append(_rest(W2[128:132] * cd))
            else:
                if cds_uniform:
                    wmain_blocks.append(W1 - cval * W2)
                    wmain_blocks.append(W2 * cval)
                else:
                    wmain_blocks.append(W1)
                    for cd in cds:
                        wmain_blocks.append(W2 * cd)
            bc = np.zeros(P, np.float32)
            bc[0:dout] = b
            bias_cols.append(bc)
        Wf, bf = params[s]["fc"]
        wfc_cols.append(np.asarray(Wf, np.float32))
        bfc_cols.append(np.asarray(bf, np.float32))

    wmain = np.concatenate(wmain_blocks, axis=1).astype(ml_dtypes.bfloat16)
    wmain = np.ascontiguousarray(wmain)
    wrest = np.concatenate(wrest_blocks, axis=1).astype(ml_dtypes.bfloat16)
    wrest = np.ascontiguousarray(wrest)
    wfc = np.concatenate(wfc_cols, axis=1).astype(ml_dtypes.bfloat16)
    bias = np.stack(bias_cols, axis=1).astype(np.float32)
    bfc = np.stack(bfc_cols, axis=1).astype(np.float32)
    return {
        "wmain": wmain,
        "wrest": np.ascontiguousarray(wrest),
        "wfc": np.ascontiguousarray(wfc),
        "bias": np.ascontiguousarray(bias),
        "bfc": np.ascontiguousarray(bfc),
    }


def _adjacency_coeffs(adjacent):
    """Verify banded-circulant structure, return (uniform, cval, cds)."""
    adjacent = np.asarray(adjacent, np.float32)
    cds = [float(adjacent[0, d % N]) for d in [-4, -3, -2, -1, 1, 2, 3, 4]]
    expect = np.zeros((N, N), np.float32)
    idx = np.arange(N)
    for d, cd in zip([-4, -3, -2, -1, 1, 2, 3, 4], cds):
        expect[idx, (idx + d) % N] = cd
    if not np.allclose(adjacent, expect, atol=1e-6):
        import warnings

        warnings.warn("adjacent is not the expected banded circulant; result may be wrong")
    uniform = max(cds) - min(cds) < 1e-9
    return uniform, cds[0], cds


def kernel(tg2, feature_hull, original_hull, binary_hull, bbox, adjacent, params):
    from concourse.bass_utils import run_bass_kernel_spmd

    uniform, cval, cds = _adjacency_coeffs(adjacent)

    key = ("prog", uniform)
    if key not in _cache:
        _cache[key] = _build_program(uniform)
    nc = _cache[key]

    in_maps = _host_prep(tg2, original_hull, binary_hull, bbox, params)
    wblobs = _pack_weights(params, uniform, cval, cds)
    for m in in_maps:
        m.update(wblobs)

    res = run_bass_kernel_spmd(nc, in_maps, core_ids=list(range(NCORES)))
    kernel._last_results = res

    preds = np.zeros((STEPS, B, N, 2), np.float32)
    for b in range(B):
        out = res.results[2 * b]["preds"]  # [STEPS, 2, N]
        preds[:, b, :, 0] = out[:, 0, :]
        preds[:, b, :, 1] = out[:, 1, :]
    return preds


# revision 5
# speedup vs baseline: 10.3053x; 10.3053x over previous
"""PolyGNN Trainium2 kernel.

Strategy (8 NeuronCores, B=4):
  - core c works on batch c // 2 (pair-replicated in v1).
  - GCN runs in transposed layout x^T [C, N] per batch, N=1024 columns with a
    4-column cyclic halo so the ring-adjacency matvec adj@x becomes 8
    accumulating PE matmuls with free-dim-shifted rhs windows (adjacent is a
    banded circulant: vertex i <-> i+-1..4 mod N, weights 1/8).
  - Bilinear CNN-feature interpolation: index/weight math on DVE in a
    [128, 8]-chunked vertex layout, rows gathered from a host-pretransposed
    DRAM table [HW, 256] bf16 via dma_gather, weighted sum on DVE, then PE
    transposes into the [C, N] GCN input.
"""

import sys

sys.path.insert(0, "/opt/trn_rl_repo")
sys.path.insert(0, "/root/.axon_site/_ro/trn_rl_repo")

import numpy as np
import ml_dtypes

B, N, FH, FW, C = 4, 1024, 128, 128, 130
SDIM, STEPS = 128, 3
HAL = 4           # cyclic halo columns on each side
XW = N + 2 * HAL  # 1032
ELEM = 256        # padded gather row length (bf16 -> 512B, %256B ok)
NCORES = 8
P = 128

_cache = {}


def _build_program(n_diag_blocks_uniform: bool, timing_repeat: int = 1):
    import concourse.bass as bass
    import concourse.bacc as bacc
    import concourse.tile as tile
    from concourse import mybir

    f32 = mybir.dt.float32
    bf16 = mybir.dt.bfloat16
    i16 = mybir.dt.int16
    i32 = mybir.dt.int32
    ALU = mybir.AluOpType
    ACTF = mybir.ActivationFunctionType

    DS = [-4, -3, -2, -1, 1, 2, 3, 4]

    # weight blob layout (must mirror host packing)
    # per step: gcn0 [W1(128), W2(128 or 8x128)], 12x res [W1, W2...], gcn7 [W1(32), W2(32...)]
    nwb = 1 if n_diag_blocks_uniform else 8

    def gconv_meta():
        # (dout, has_rest, relu, residual)
        metas = [(SDIM, True, False, False)]
        for _ in range(6):
            metas.append((SDIM, False, True, False))
            metas.append((SDIM, False, True, True))
        metas.append((32, False, False, False))
        return metas

    metas = gconv_meta()
    # compute main-blob column offsets
    offs = []
    cur = 0
    for s in range(STEPS):
        step_offs = []
        for dout, has_rest, relu, resid in metas:
            w1o = cur
            cur += dout
            w2o = [cur + (i if not n_diag_blocks_uniform else 0) * dout for i in range(8)]
            cur += dout * nwb
            step_offs.append((w1o, w2o))
        offs.append(step_offs)
    NW = cur
    # rest blob: per step W1rest(128) + W2rest blocks
    roffs = []
    cur = 0
    for s in range(STEPS):
        r1 = cur
        cur += SDIM
        r2 = [cur + (i if not n_diag_blocks_uniform else 0) * SDIM for i in range(8)]
        cur += SDIM * nwb
        roffs.append((r1, r2))
    NR = cur

    nc = bacc.Bacc(None, target_bir_lowering=False, debug=False)

    conv_d = nc.dram_tensor("conv", [FH * FW, ELEM], bf16, kind="ExternalInput")
    xy0_d = nc.dram_tensor("xy0", [P, 16], f32, kind="ExternalInput")
    hull0_d = nc.dram_tensor("hull0", [2, N], f32, kind="ExternalInput")
    wmain_d = nc.dram_tensor("wmain", [P, NW], bf16, kind="ExternalInput")
    wrest_d = nc.dram_tensor("wrest", [64, NR], bf16, kind="ExternalInput")
    wfc_d = nc.dram_tensor("wfc", [32, 2 * STEPS], bf16, kind="ExternalInput")
    bias_d = nc.dram_tensor("bias", [P, 14 * STEPS], f32, kind="ExternalInput")
    bfc_d = nc.dram_tensor("bfc", [2, STEPS], f32, kind="ExternalInput")
    ident_d = nc.dram_tensor("ident", [P, P], bf16, kind="ExternalInput")
    identf_d = nc.dram_tensor("identf", [P, P], f32, kind="ExternalInput")
    r16_d = nc.dram_tensor("r16", [16, P], f32, kind="ExternalInput")
    preds_d = nc.dram_tensor("preds", [STEPS, 2, N], f32, kind="ExternalOutput")

    with tile.TileContext(nc) as tc:
        with (
            tc.tile_pool(name="persist", bufs=1) as pp,
            tc.tile_pool(name="interp", bufs=2) as ip,
            tc.tile_pool(name="gpool", bufs=1) as gp,
            tc.tile_pool(name="xpool", bufs=4) as xp,
            tc.tile_pool(name="psmm", bufs=4, space="PSUM") as psmm,
            tc.tile_pool(name="pstr", bufs=2, space="PSUM") as pstr,
        ):
            # ---- load persistent data
            wmain_t = pp.tile([P, NW], bf16)
            nc.sync.dma_start(wmain_t[:], wmain_d[:])
            wrest_t = pp.tile([64, NR], bf16)
            nc.sync.dma_start(wrest_t[:], wrest_d[:])
            wfc_t = pp.tile([32, 2 * STEPS], bf16)
            nc.sync.dma_start(wfc_t[:], wfc_d[:])
            bias_t = pp.tile([P, 14 * STEPS], f32)
            nc.sync.dma_start(bias_t[:], bias_d[:])
            bfc_t = pp.tile([2, STEPS], f32)
            nc.sync.dma_start(bfc_t[:], bfc_d[:])
            ident_t = pp.tile([P, P], bf16)
            nc.sync.dma_start(ident_t[:], ident_d[:])
            identf_t = pp.tile([P, P], f32)
            nc.sync.dma_start(identf_t[:], identf_d[:])
            r16_t = pp.tile([16, P], f32)
            nc.sync.dma_start(r16_t[:], r16_d[:])
            xy0_t = pp.tile([P, 16], f32)
            nc.sync.dma_start(xy0_t[:], xy0_d[:])
            hull_t = pp.tile([2, N], f32)
            nc.sync.dma_start(hull_t[:], hull0_d[:])

            for _rep in range(timing_repeat):
              for s in range(STEPS):
                # ================= interp: S = Xs/Ys in [128, (j,c)] chunk layout
                if s == 0:
                    s_t = xy0_t
                else:
                    hc_t = ip.tile([P, 16], f32, tag="hc")
                    for j in range(8):
                        trp = pstr.tile([P, 2], f32, space="PSUM", tag="pstr")
                        nc.tensor.transpose(
                            trp[:], hull_t[:, 128 * j : 128 * j + 128],
                            identf_t[0:2, 0:2],
                        )
                        nc.vector.tensor_copy(hc_t[:, 2 * j : 2 * j + 2], trp[:])
                    s_t = ip.tile([P, 16], f32, tag="s")
                    nc.vector.tensor_scalar(
                        out=s_t[:], in0=hc_t[:], scalar1=128.0, scalar2=None,
                        op0=ALU.mult,
                    )

                # floor & frac (rounding-mode agnostic)
                ri_t = ip.tile([P, 16], i32, tag="ri")
                nc.vector.tensor_copy(ri_t[:], s_t[:])
                rf_t = ip.tile([P, 16], f32, tag="rf")
                nc.vector.tensor_copy(rf_t[:], ri_t[:])
                cm_t = ip.tile([P, 16], f32, tag="cm")
                nc.vector.tensor_tensor(out=cm_t[:], in0=s_t[:], in1=rf_t[:], op=ALU.is_lt)
                f0_t = ip.tile([P, 16], f32, tag="f0")
                nc.vector.tensor_tensor(out=f0_t[:], in0=rf_t[:], in1=cm_t[:], op=ALU.subtract)
                fr_t = ip.tile([P, 16], f32, tag="fr")
                nc.vector.tensor_tensor(out=fr_t[:], in0=s_t[:], in1=f0_t[:], op=ALU.subtract)
                f0c_t = ip.tile([P, 16], f32, tag="f0c")
                nc.vector.tensor_scalar(
                    out=f0c_t[:], in0=f0_t[:], scalar1=0.0, scalar2=127.0,
                    op0=ALU.max, op1=ALU.min,
                )
                t1_t = ip.tile([P, 16], f32, tag="t1")
                nc.vector.tensor_scalar(
                    out=t1_t[:], in0=f0_t[:], scalar1=1.0, scalar2=0.0,
                    op0=ALU.add, op1=ALU.max,
                )
                f1c_t = ip.tile([P, 16], f32, tag="f1c")
                nc.vector.tensor_scalar(
                    out=f1c_t[:], in0=t1_t[:], scalar1=127.0, scalar2=None, op0=ALU.min,
                )
                om_t = ip.tile([P, 16], f32, tag="om")
                nc.vector.tensor_scalar(
                    out=om_t[:], in0=fr_t[:], scalar1=1.0, scalar2=-1.0,
                    op0=ALU.subtract, op1=ALU.mult,
                )

                def xy(t):
                    v = t[:].rearrange("p (j c) -> p j c", c=2)
                    return v[:, :, 0], v[:, :, 1]

                frx, fry = xy(fr_t)
                omx, omy = xy(om_t)
                f0cx, f0cy = xy(f0c_t)
                f1cx, f1cy = xy(f1c_t)

                # bilinear weights, bf16 [128, 8] each
                w_ts = []
                for wi, (ax, ay) in enumerate(
                    [(omx, omy), (omx, fry), (frx, omy), (frx, fry)]
                ):
                    w_t = ip.tile([P, 8], bf16, tag=f"w{wi}")
                    nc.vector.tensor_tensor(out=w_t[:], in0=ax, in1=ay, op=ALU.mult)
                    w_ts.append(w_t)

                # gather row indices (fp32), nbr-major [128, (i, j)]
                a0_t = ip.tile([P, 8], f32, tag="a0")
                nc.vector.tensor_scalar(
                    out=a0_t[:], in0=f0cx, scalar1=float(FW), scalar2=None, op0=ALU.mult
                )
                a1_t = ip.tile([P, 8], f32, tag="a1")
                nc.vector.tensor_scalar(
                    out=a1_t[:], in0=f1cx, scalar1=float(FW), scalar2=None, op0=ALU.mult
                )
                idxall_t = ip.tile([P, 32], f32, tag="idxall")
                for i, (ax, ay) in enumerate(
                    [(a0_t, f0cy), (a0_t, f1cy), (a1_t, f0cy), (a1_t, f1cy)]
                ):
                    axap = ax[:] if isinstance(ax, type(a0_t)) else ax
                    nc.vector.tensor_tensor(
                        out=idxall_t[:, 8 * i : 8 * i + 8], in0=axap, in1=ay, op=ALU.add
                    )

                # ---- convert to dma_gather wrapped-idx layout [16, 64] x4, replicated
                tps = pstr.tile([32, P], f32, space="PSUM", tag="pstr")
                nc.tensor.transpose(tps[:], idxall_t[:], identf_t[:])
                u_t = ip.tile([32, P], f32, tag="u")
                nc.vector.tensor_copy(u_t[:], tps[:])
                idx16f_t = ip.tile([16, 256], f32, tag="idx16f")
                for a in range(8):
                    uap = pstr.tile([16, 32], f32, space="PSUM", tag="pstr")
                    nc.tensor.transpose(
                        uap[:], u_t[:, 16 * a : 16 * a + 16], identf_t[0:32, 0:32]
                    )
                    dst = bass.AP(
                        idx16f_t.tensor,
                        idx16f_t[:].offset + a,
                        [[idx16f_t[:].ap[0][0], 16], [64, 4], [8, 8]],
                    )
                    nc.vector.tensor_copy(dst, uap[:])
                repp = pstr.tile([P, 256], f32, space="PSUM", tag="pstr")
                nc.tensor.matmul(
                    repp[:], lhsT=r16_t[:], rhs=idx16f_t[:], start=True, stop=True
                )
                idx16_t = ip.tile([P, 256], i16, tag="idx16")
                nc.vector.tensor_copy(idx16_t[:], repp[:])

                # ---- gathers
                g_ts = []
                for i in range(4):
                    g_t = gp.tile([P, 8, ELEM], bf16, tag=f"g{i}")
                    nc.gpsimd.dma_gather(
                        g_t[:],
                        conv_d[:],
                        idx16_t[:, 64 * i : 64 * i + 64],
                        N,
                        N,
                        ELEM,
                    )
                    g_ts.append(g_t)

                # ---- weighted sum -> cnn [128, 8, 130] bf16
                def wbc(w_t):
                    ap = w_t[:]
                    return bass.AP(ap.tensor, ap.offset, [[ap.ap[0][0], P], [1, 8], [0, C]])

                m_ts = []
                for i in range(4):
                    m_t = ip.tile([P, 8, C], bf16, tag=f"m{i % 2}_{i // 2}")
                    nc.vector.tensor_tensor(
                        out=m_t[:], in0=g_ts[i][:, :, 0:C], in1=wbc(w_ts[i]), op=ALU.mult
                    )
                    m_ts.append(m_t)
                s01_t = ip.tile([P, 8, C], bf16, tag="s01")
                nc.vector.tensor_tensor(out=s01_t[:], in0=m_ts[0][:], in1=m_ts[1][:], op=ALU.add)
                s23_t = ip.tile([P, 8, C], bf16, tag="s23")
                nc.vector.tensor_tensor(out=s23_t[:], in0=m_ts[2][:], in1=m_ts[3][:], op=ALU.add)
                cnn_t = ip.tile([P, 8, C], bf16, tag="cnn")
                nc.vector.tensor_tensor(out=cnn_t[:], in0=s01_t[:], in1=s23_t[:], op=ALU.add)

                # ---- transpose into GCN input x^T
                x_m = xp.tile([P, XW], bf16, tag="xm")
                x_r = xp.tile([64, XW], bf16, tag="xr")
                nc.vector.memset(x_r[:], 0.0)
                for j in range(8):
                    tmp = pstr.tile([P, P], bf16, space="PSUM", tag="pstr")
                    nc.tensor.transpose(tmp[:], cnn_t[:, j, 0:128], ident_t[:])
                    nc.scalar.copy(x_m[:, HAL + 128 * j : HAL + 128 * j + 128], tmp[:])
                    tmp2 = pstr.tile([2, P], bf16, space="PSUM", tag="pstr")
                    nc.tensor.transpose(tmp2[:], cnn_t[:, j, 128:130], ident_t[:])
                    nc.vector.tensor_copy(
                        x_r[0:2, HAL + 128 * j : HAL + 128 * j + 128], tmp2[:]
                    )
                nc.vector.tensor_copy(x_r[32:34, HAL : HAL + N], hull_t[:])
                # halos
                nc.vector.tensor_copy(x_m[:, 0:HAL], x_m[:, N : N + HAL])
                nc.vector.tensor_copy(x_m[:, N + HAL : N + 2 * HAL], x_m[:, HAL : 2 * HAL])
                nc.vector.tensor_copy(x_r[:, 0:HAL], x_r[:, N : N + HAL])
                nc.vector.tensor_copy(x_r[:, N + HAL : N + 2 * HAL], x_r[:, HAL : 2 * HAL])

                # ---- GCN
                xres_m = None
                for g, (dout, has_rest, relu, resid) in enumerate(metas):
                    w1o, w2o = offs[s][g]
                    y_m = xp.tile([dout, XW] if dout != SDIM else [P, XW], bf16, tag="xm")
                    if n_diag_blocks_uniform:
                        # spread-3: s = x<<-3 + x + x<<3 on DVE; then
                        # psum = x@(W1-cW2) + box3(s)@(cW2)
                        s_m = xp.tile([P, XW], bf16, tag="sm")
                        nc.vector.tensor_tensor(
                            out=s_m[:, 3 : XW - 3], in0=x_m[:, 0 : XW - 6],
                            in1=x_m[:, 6:XW], op=ALU.add,
                        )
                        nc.vector.tensor_tensor(
                            out=s_m[:, 3 : XW - 3], in0=s_m[:, 3 : XW - 3],
                            in1=x_m[:, 3 : XW - 3], op=ALU.add,
                        )
                        if has_rest:
                            s_r = xp.tile([64, XW], bf16, tag="sr")
                            nc.vector.tensor_tensor(
                                out=s_r[:, 3 : XW - 3], in0=x_r[:, 0 : XW - 6],
                                in1=x_r[:, 6:XW], op=ALU.add,
                            )
                            nc.vector.tensor_tensor(
                                out=s_r[:, 3 : XW - 3], in0=s_r[:, 3 : XW - 3],
                                in1=x_r[:, 3 : XW - 3], op=ALU.add,
                            )
                    for h in range(2):
                        base = HAL + 512 * h
                        pshalf = psmm.tile([dout, 512], f32, space="PSUM", tag="psum")
                        mms = [(wmain_t[:, w1o : w1o + dout], x_m[:, base : base + 512])]
                        if has_rest:
                            r1, r2 = roffs[s]
                            mms.append(
                                (wrest_t[:, r1 : r1 + SDIM], x_r[:, base : base + 512])
                            )
                        if n_diag_blocks_uniform:
                            for d in (-1, 0, 1):
                                mms.append(
                                    (
                                        wmain_t[:, w2o[0] : w2o[0] + dout],
                                        s_m[:, base + d : base + d + 512],
                                    )
                                )
                                if has_rest:
                                    r1, r2 = roffs[s]
                                    mms.append(
                                        (
                                            wrest_t[:, r2[0] : r2[0] + SDIM],
                                            s_r[:, base + d : base + d + 512],
                                        )
                                    )
                        else:
                            for di, d in enumerate(DS):
                                mms.append(
                                    (
                                        wmain_t[:, w2o[di] : w2o[di] + dout],
                                        x_m[:, base + d : base + d + 512],
                                    )
                                )
                                if has_rest:
                                    r1, r2 = roffs[s]
                                    mms.append(
                                        (
                                            wrest_t[:, r2[di] : r2[di] + SDIM],
                                            x_r[:, base + d : base + d + 512],
                                        )
                                    )
                        if resid:
                            mms.append((ident_t[:], xres_m[:, base : base + 512]))
                        for mi, (lhsT_ap, rhs_ap) in enumerate(mms):
                            nc.tensor.matmul(
                                pshalf[:],
                                lhsT=lhsT_ap,
                                rhs=rhs_ap,
                                start=(mi == 0),
                                stop=(mi == len(mms) - 1),
                            )
                        # evacuate with bias (+relu)
                        nc.scalar.activation(
                            y_m[0:dout, base : base + 512],
                            pshalf[:],
                            ACTF.Relu if relu else ACTF.Identity,
                            bias=bias_t[0:dout, 14 * s + g : 14 * s + g + 1],
                            scale=1.0,
                        )
                    # halos
                    nc.vector.tensor_copy(y_m[0:dout, 0:HAL], y_m[0:dout, N : N + HAL])
                    nc.vector.tensor_copy(
                        y_m[0:dout, N + HAL : N + 2 * HAL], y_m[0:dout, HAL : 2 * HAL]
                    )
                    if g + 1 < len(metas) and metas[g + 1][3]:
                        xres_m = x_m  # input of gconv_a feeds residual of gconv_b
                    x_m = y_m

                # ---- fc + hull update + snapshot
                for h in range(2):
                    base = HAL + 512 * h
                    pf = psmm.tile([2, 512], f32, space="PSUM", tag="psum")
                    nc.tensor.matmul(
                        pf[:],
                        lhsT=wfc_t[:, 2 * s : 2 * s + 2],
                        rhs=x_m[0:32, base : base + 512],
                        start=True,
                        stop=True,
                    )
                    sh_t = ip.tile([2, 512], f32, tag="sh")
                    nc.scalar.activation(
                        sh_t[:], pf[:], ACTF.Identity,
                        bias=bfc_t[:, s : s + 1], scale=1.0,
                    )
                    nc.vector.tensor_tensor(
                        out=hull_t[:, 512 * h : 512 * h + 512],
                        in0=hull_t[:, 512 * h : 512 * h + 512],
                        in1=sh_t[:],
                        op=ALU.add,
                    )
                nc.sync.dma_start(preds_d[s], hull_t[:])

    nc.compile()
    return nc


_RESID_FIX_NOTE = """residual handled via identity matmul"""


def _host_prep(tg2, original_hull, binary_hull, bbox, params):
    """Build per-core input maps."""
    tg2 = np.asarray(tg2, np.float32)
    original_hull = np.asarray(original_hull, np.float32)
    binary_hull = np.asarray(binary_hull, np.float32)
    bbox = np.asarray(bbox, np.float32)

    ident = np.eye(P, dtype=ml_dtypes.bfloat16)
    identf = np.eye(P, dtype=np.float32)
    r16 = np.zeros((16, P), np.float32)
    for q in range(P):
        r16[q % 16, q] = 1.0

    in_maps = []
    for core in range(NCORES):
        b = core // 2
        conv = np.zeros((FH * FW, ELEM), ml_dtypes.bfloat16)
        conv[:, 0:C] = (
            tg2[b].transpose(1, 2, 0).reshape(FH * FW, C).astype(ml_dtypes.bfloat16)
        )
        h, w = bbox[b, 3], bbox[b, 2]
        s0 = np.empty((N, 2), np.float32)
        s0[:, 0] = original_hull[b, :, 0] / h * FH
        s0[:, 1] = original_hull[b, :, 1] / w * FW
        # chunk layout [p, (j, c)]
        xy0 = s0.reshape(8, 128, 2).transpose(1, 0, 2).reshape(P, 16).copy()
        hull0 = np.ascontiguousarray(binary_hull[b].T)
        in_maps.append(
            {
                "conv": conv,
                "xy0": xy0,
                "hull0": hull0,
                "ident": ident,
                "identf": identf,
                "r16": r16,
            }
        )
    return in_maps


def _pack_weights(params, cds_uniform, cval, cds):
    """Pack per-step GCN weights into blobs. Returns dict of arrays."""
    nwb = 1 if cds_uniform else 8
    wmain_blocks = []
    wrest_blocks = []
    wfc_cols = []
    bias_cols = []
    bfc_cols = []
    for s in range(STEPS):
        p = params[s]
        gconvs = [p["gcn0"]]
        for pa, pb in p["res"]:
            gconvs.append(pa)
            gconvs.append(pb)
        gconvs.append(p["gcn7"])
        for gi, (W1, b1, W2, b2) in enumerate(gconvs):
            W1 = np.asarray(W1, np.float32)
            W2 = np.asarray(W2, np.float32)
            b = np.asarray(b1, np.float32) + np.asarray(b2, np.float32)
            dout = W1.shape[1]
            if gi == 0:
                def _rest(Wsub):
                    r = np.zeros((64, Wsub.shape[1]), np.float32)
                    r[0:2] = Wsub[0:2]    # cnn channels 128, 129
                    r[32:34] = Wsub[2:4]  # hull x, y rows
                    return r
                if cds_uniform:
                    # spread-3: adj@x = c*(box3(s) - x), s = x<<-3 + x + x<<3
                    wmain_blocks.append(W1[0:128] - cval * W2[0:128])
                    wrest_blocks.append(_rest(W1[128:132] - cval * W2[128:132]))
                    wmain_blocks.append(W2[0:128] * cval)
                    wrest_blocks.append(_rest(W2[128:132] * cval))
                else:
                    wmain_blocks.append(W1[0:128])
                    wrest_blocks.append(_rest(W1[128:132]))
                    for cd in cds:
                        wmain_blocks.append(W2[0:128] * cd)
                        wrest_blocks.append(_rest(W2[128:132] * cd))
            else:
                if cds_uniform:
                    wmain_blocks.append(W1 - cval * W2)
                    wmain_blocks.append(W2 * cval)
                else:
                    wmain_blocks.append(W1)
                    for cd in cds:
                        wmain_blocks.append(W2 * cd)
            bc = np.zeros(P, np.float32)
            bc[0:dout] = b
            bias_cols.append(bc)
        Wf, bf = params[s]["fc"]
        wfc_cols.append(np.asarray(Wf, np.float32))
        bfc_cols.append(np.asarray(bf, np.float32))

    wmain = np.concatenate(wmain_blocks, axis=1).astype(ml_dtypes.bfloat16)
    wmain = np.ascontiguousarray(wmain)
    wrest = np.concatenate(wrest_blocks, axis=1).astype(ml_dtypes.bfloat16)
    wrest = np.ascontiguousarray(wrest)
    wfc = np.concatenate(wfc_cols, axis=1).astype(ml_dtypes.bfloat16)
    bias = np.stack(bias_cols, axis=1).astype(np.float32)
    bfc = np.stack(bfc_cols, axis=1).astype(np.float32)
    return {
        "wmain": wmain,
        "wrest": np.ascontiguousarray(wrest),
        "wfc": np.ascontiguousarray(wfc),
        "bias": np.ascontiguousarray(bias),
        "bfc": np.ascontiguousarray(bfc),
    }


def _adjacency_coeffs(adjacent):
    """Verify banded-circulant structure, return (uniform, cval, cds)."""
    adjacent = np.asarray(adjacent, np.float32)
    cds = [float(adjacent[0, d % N]) for d in [-4, -3, -2, -1, 1, 2, 3, 4]]
    expect = np.zeros((N, N), np.float32)
    idx = np.arange(N)
    for d, cd in zip([-4, -3, -2, -1, 1, 2, 3, 4], cds):
        expect[idx, (idx + d) % N] = cd
    if not np.allclose(adjacent, expect, atol=1e-6):
        import warnings

        warnings.warn("adjacent is not the expected banded circulant; result may be wrong")
    uniform = max(cds) - min(cds) < 1e-9
    return uniform, cds[0], cds


def kernel(tg2, feature_hull, original_hull, binary_hull, bbox, adjacent, params):
    from concourse.bass_utils import run_bass_kernel_spmd

    uniform, cval, cds = _adjacency_coeffs(adjacent)

    key = ("prog", uniform)
    if key not in _cache:
        _cache[key] = _build_program(uniform)
    nc = _cache[key]

    in_maps = _host_prep(tg2, original_hull, binary_hull, bbox, params)
    wblobs = _pack_weights(params, uniform, cval, cds)
    for m in in_maps:
        m.update(wblobs)

    res = run_bass_kernel_spmd(nc, in_maps, core_ids=list(range(NCORES)))
    kernel._last_results = res

    preds = np.zeros((STEPS, B, N, 2), np.float32)
    for b in range(B):
        out = res.results[2 * b]["preds"]  # [STEPS, 2, N]
        preds[:, b, :, 0] = out[:, 0, :]
        preds[:, b, :, 1] = out[:, 1, :]
    return preds
